# revision 40
# baseline (speedup 1.0000x reference)
"""MoE layer (top-2 routing, SwiGLU experts) on 8 TRN2 NeuronCores.

Strategy (expert-parallel, matching the sharding hint):
  - Host computes the router (logits -> top-2 -> softmax weights) in f64
    numpy. This is the dispatch decision of the all-to-all; it is ~0.05%
    of the FLOPs. The min gap between the 2nd and 3rd logit is ~1.1e-4,
    so f64 routing agrees with the fp32 reference's selection.
  - Core e receives the tokens routed to expert e (gathered, transposed,
    zero-padded to a static capacity C), expert e's weights
    (pre-transposed on host), and the per-token combine weight.
  - Each core runs the expert FFN: g = x@WgT, u = x@WuT, h = silu(g)*u,
    y = (h@WdT) * combine, all matmuls on the PE array in bf16 with f32
    PSUM accumulation (bf16 and f32r run at the same PE rate, but bf16
    halves the HBM weight traffic that stalled the f32r version).
  - Host scatter-adds each expert's scaled output rows into the full
    [T, H] output (the combine of the all-to-all).

Kernel layout per core (C = token capacity, single chunk-group so the
Wg/Wu stripes stream exactly once, all in bf16):
  pass1 (per i-tile): g-phase then u-phase. Each phase sweeps the 8
    h-tiles; per h one stationary weight block serves all C moving
    columns (512-greedy chunks into one bank-aligned PSUM tile per
    matrix). ACT drains silu(g) into hh during the u-phase; DVE
    multiplies u in during the next i's g-phase, so single-buffered
    PSUM (6 banks) pipelines without PE stalls. Redundant LDWEIGHTS
    (same stationary across consecutive matmuls) are deduped post-
    schedule. Resident WdT rows are DMA'd interleaved with the stripe
    loads so they never delay pass 1.
  pass2 (per 128-token tile): two 512-wide PSUM chains accumulate over
    22 i-tiles with stationary hh tiles and moving resident-WdT rows;
    DVE applies the per-partition combine scale (tensor_scalar_mul)
    into SBUF -> DRAM, token-major [C, H]. PSUM ring (2 bufs) overlaps
    drains with the next tile's chains.
"""

import sys

if "/opt/trn_rl_repo" not in sys.path:
    sys.path.insert(0, "/opt/trn_rl_repo")

import numpy as np
import ml_dtypes

BF16 = np.dtype(ml_dtypes.bfloat16)

B, S, H, I, E = 2, 2048, 1024, 2816, 8
T = B * S
HT = H // 128   # 8 h-tiles
IT = I // 128   # 22 i-tiles
TOP_K = 2

_PROG_CACHE = {}


def _split_waits(nc):
    """This walrus build rejects >1 sync wait per instruction; move extra
    waits onto standalone event-sem instructions on the issuing engine.
    For HWDGE DMAs the enqueue happens at engine-execution time, so a
    preceding engine-stream wait still gates the transfer."""
    import concourse.mybir as mybir

    for f in nc.m.functions:
        for blk in f.blocks:
            out = []
            for inst in blk.instructions:
                si = inst.sync_info
                if si is None or len(si.on_wait) <= 1:
                    out.append(inst)
                    continue
                waits = list(si.on_wait)
                for k, w in enumerate(waits[:-1]):
                    ev = mybir.InstEventSemaphore(name=f"{inst.name}_ws{k}")
                    ev.engine = inst.engine
                    ev.sync_info = mybir.SyncInfo(on_wait=[w], on_update=[])
                    out.append(ev)
                while len(si.on_wait) > 1:
                    si.on_wait.pop(0)
                out.append(inst)
            blk.instructions = out


def _dedup_ldweights(nc):
    """Drop InstLdweights whose weight AP is identical to the previous
    LDWEIGHTS on the PE stream (the PE array still holds those weights).
    The Tile pipeline emits one LDWEIGHTS per matmul with no dedup, so
    stationary-operand reuse across consecutive matmuls is otherwise
    wasted. Sem waits/updates of a dropped LDW move to the next PE
    instruction. Must run before _split_waits."""
    import concourse.mybir as mybir

    for f in nc.m.functions:
        for blk in f.blocks:
            last_sig = None
            drop = []
            pending = None  # sync_info carried from a dropped LDW
            out = []
            for inst in blk.instructions:
                if getattr(inst, "engine", None) != mybir.EngineType.PE:
                    out.append(inst)
                    continue
                if isinstance(inst, mybir.InstLdweights):
                    sig = str(inst.ins[0])
                    if sig == last_sig:
                        si = inst.sync_info
                        if si is not None and (si.on_wait or si.on_update):
                            if pending is None:
                                pending = ([], [])
                            pending[0].extend(si.on_wait)
                            pending[1].extend(si.on_update)
                        continue  # drop this redundant LDW
                    last_sig = sig
                elif not isinstance(inst, mybir.InstMatmult):
                    # any other PE instruction: be conservative
                    last_sig = None
                if pending is not None:
                    w, u = pending
                    if inst.sync_info is None:
                        inst.sync_info = mybir.SyncInfo(on_wait=[], on_update=[])
                    inst.sync_info.on_wait.extend(w)
                    inst.sync_info.on_update.extend(u)
                    pending = None
                out.append(inst)
            blk.instructions = out


WBUFS = 6              # wg/wu stripe prefetch depth (3 i-tiles)
OUTBUFS = 4            # out staging depth
Y_BF16 = False         # bf16 output measured slower (DVE f32->bf16 drain
                       # loses its fast path); keep f32 out
PSUM_TIGHT = False     # dead end: the PSUM allocator rounds tiles up to
                       # 2KB banks, so sub-bank packing can't free a bank
DEDUP_LDW = True       # drop redundant LDWEIGHTS (same stationary reuse)
FUSE_DRAIN = False     # split ACT/DVE drains per chunk (measured faster)
TAIL_SPLIT = False     # interleaved tail matmuls (measured faster)
PASS_FILTER = None     # None | "p1" | "p2"  (diagnostics only)
P2_TILES = None        # diagnostics: limit p2 to first N token tiles
P1_TILES = None        # diagnostics: limit p1 to first N i-tiles


def _chunks_of(C):
    """Split C (multiple of 128) into matmul-N chunks: greedy 512s plus a
    remainder chunk, so each LDWEIGHTS is amortized over max-N matmuls."""
    out = []
    c0 = 0
    rem = C
    while rem > 0:
        cn = min(512, rem)
        out.append((c0, cn))
        c0 += cn
        rem -= cn
    return out


def _build_program(C, repeat=1, bench=False):
    import concourse.bass as bass
    import concourse.mybir as mybir
    from concourse.tile import TileContext

    dt = mybir.dt
    f32 = dt.float32
    bf16 = dt.bfloat16
    Silu = mybir.ActivationFunctionType.Silu
    CT = C // 128
    chunks = _chunks_of(C)
    # PSUM budget: pass-1 uses one C-column tile per matrix (matmul
    # outputs may cross bank boundaries); pass-2 gets the rest (<=3).
    _p1_bytes = 2 * (C if PSUM_TIGHT else 512 * len(chunks)) * 4
    _p2_bufs = max(1, min(3, (16384 - _p1_bytes) // 2048))

    nc = bass.Bass()
    if bench:
        # timing-only build: big tensors live in internal DRAM (no host
        # transfer); only a tiny dummy output is external
        xT = nc.dram_tensor("xT", [H, C], bf16)
        wg = nc.dram_tensor("wg", [IT, 128, H], bf16)
        wu = nc.dram_tensor("wu", [IT, 128, H], bf16)
        wd = nc.dram_tensor("wd", [I, H], bf16)
        ce = nc.dram_tensor("ce", [128, CT], f32)
        y = nc.dram_tensor("y", [C, H], bf16 if Y_BF16 else f32)
        dummy = nc.declare_dram_parameter("bench_out", [128, 4], f32, isOutput=True)
    else:
        xT = nc.declare_dram_parameter("xT", [H, C], bf16, isOutput=False)
        wg = nc.declare_dram_parameter("wg", [IT, 128, H], bf16, isOutput=False)
        wu = nc.declare_dram_parameter("wu", [IT, 128, H], bf16, isOutput=False)
        wd = nc.declare_dram_parameter("wd", [I, H], bf16, isOutput=False)
        ce = nc.declare_dram_parameter("ce", [128, CT], f32, isOutput=False)
        y = nc.declare_dram_parameter(
            "y", [C, H], bf16 if Y_BF16 else f32, isOutput=True)

    wd_r = wd.rearrange("(it p) hd -> p it hd", p=128)
    xT_r = xT.rearrange("(ht p) c -> p ht c", p=128)

    with TileContext(nc) as tc:
        with (
            tc.tile_pool(name="resident", bufs=1) as resident,
            tc.tile_pool(name="wstripe", bufs=WBUFS) as wstripe,
            tc.tile_pool(name="xtp", bufs=1) as xtpool,
            tc.tile_pool(name="hh", bufs=1) as hhpool,
            tc.tile_pool(name="outp", bufs=OUTBUFS) as outp,
            tc.tile_pool(name="ps1", bufs=1, space="PSUM") as ps1,
            tc.tile_pool(name="ps2", bufs=_p2_bufs, space="PSUM") as ps2,
        ):
            if bench:
                # zero-fill internal tensors so timing data is clean fp
                ztf = outp.tile([128, H], f32, tag="out")
                nc.vector.memset(ztf[:, :], 0.0)
                zt = resident.tile([128, C], bf16, tag="z16")
                nc.vector.memset(zt[:, :], 0.0)

                for i in range(IT):
                    nc.sync.dma_start(out=wg[i, :, :], in_=zt[:, :H])
                    nc.sync.dma_start(out=wu[i, :, :], in_=zt[:, :H])
                    nc.sync.dma_start(out=wd_r[:, i, :], in_=zt[:, :H])
                for h in range(HT):
                    nc.sync.dma_start(out=xT_r[:, h, :], in_=zt[:, :C])
                nc.sync.dma_start(out=ce[:, :], in_=ztf[:, :CT])

            # Resident combine weights
            ce_sb = resident.tile([128, CT], f32)
            nc.sync.dma_start(out=ce_sb[:, :], in_=ce[:, :])
            # Resident WdT rows: DMAs issued inside the pass-1 loop below
            # (interleaved with stripe loads) so they don't delay pass 1.
            wd_sb = resident.tile([128, IT, H], bf16)

            def body():
                hh = hhpool.tile([128, IT, C], bf16, tag="hh")
                xt_sb = xtpool.tile([128, HT, C], bf16, tag="xt")
                nc.sync.dma_start(out=xt_sb[:, :, :], in_=xT_r[:, :, :])
                # ---- pass 1: hh = silu(x@WgT) * (x@WuT) ----
                # One LDWEIGHTS per (i, h, matrix) amortized over all C
                # columns (3 consecutive matmuls). g-phase then u-phase per
                # i-tile; ACT drains g during the u-phase, DVE drains u
                # during the next i's g-phase, so single-buffered PSUM
                # (6 banks) pipelines without PE stalls.
                if PASS_FILTER == "p2":
                    # diagnostics: no pass 1; fill hh so pass 2 has data
                    for i in range(IT):
                        nc.sync.dma_start(out=wd_sb[:, i, :], in_=wd_r[:, i, :])
                    nc.vector.memset(
                        hh[:, :, :].rearrange("p a b -> p (a b)"), 0.0)
                for i in range(IT if P1_TILES is None else P1_TILES):
                    if PASS_FILTER == "p2":
                        break
                    wgt = wstripe.tile([128, HT, 128], bf16, tag="wg")
                    wut = wstripe.tile([128, HT, 128], bf16, tag="wu")
                    nc.sync.dma_start(
                        out=wgt[:, :, :].rearrange("p ht c -> p (ht c)"),
                        in_=wg[i, :, :],
                    )
                    nc.sync.dma_start(
                        out=wut[:, :, :].rearrange("p ht c -> p (ht c)"),
                        in_=wu[i, :, :],
                    )
                    # interleave one resident-wd row load per i-tile
                    nc.sync.dma_start(out=wd_sb[:, i, :], in_=wd_r[:, i, :])
                    # one bank-aligned PSUM tile per matrix: matmuls hit
                    # 512-aligned slices, drain is ONE engine instruction
                    # (ACT/DVE read PSUM across banks fine)
                    for wt, act in ((wgt, True), (wut, False)):
                        t = ps1.tile(
                            [128, C if PSUM_TIGHT else 512 * len(chunks)],
                            f32, tag="g" if act else "u",
                            name="g_all" if act else "u_all")
                        if TAIL_SPLIT and chunks[-1][1] < 512:
                            phases = [chunks[:-1], chunks[-1:]]
                        else:
                            phases = [chunks]
                        for ph in phases:
                            for h in range(HT):
                                for c0, cn in ph:
                                    nc.tensor.matmul(
                                        t[:, c0:c0 + cn],
                                        wt[:, h, :],
                                        xt_sb[:, h, c0:c0 + cn],
                                        start=(h == 0),
                                        stop=(h == HT - 1),
                                    )
                        if FUSE_DRAIN:
                            hslice = hh[:, i, 0:C]
                            if act:
                                nc.scalar.activation(hslice, t[:, :C], Silu)
                            else:
                                nc.vector.tensor_mul(
                                    hslice, hslice, t[:, :C])
                        else:
                            for c0, cn in chunks:
                                hs = hh[:, i, c0:c0 + cn]
                                if act:
                                    nc.scalar.activation(
                                        hs, t[:, c0:c0 + cn], Silu)
                                else:
                                    nc.vector.tensor_mul(
                                        hs, hs, t[:, c0:c0 + cn])
                # ---- pass 2: y = (hh @ WdT) * combine ----
                p2n = CT if P2_TILES is None else P2_TILES
                for ci in range(p2n) if PASS_FILTER != "p1" else ():
                    yh = [
                        ps2.tile([128, 512], f32, tag="y", name=f"y{nh}")
                        for nh in range(2)
                    ]
                    cs = ci * 128
                    for i in range(IT):
                        for nh in range(2):
                            nc.tensor.matmul(
                                yh[nh][:, :],
                                hh[:, i, cs:cs + 128],
                                wd_sb[:, i, nh * 512:(nh + 1) * 512],
                                start=(i == 0),
                                stop=(i == IT - 1),
                            )
                    out_sb = outp.tile([128, H], bf16 if Y_BF16 else f32,
                                       tag="out")
                    for nh in range(2):
                        nc.vector.tensor_scalar_mul(
                            out_sb[:, nh * 512:(nh + 1) * 512],
                            yh[nh][:, :],
                            ce_sb[:, ci:ci + 1],
                        )
                    nc.sync.dma_start(out=y[cs:cs + 128, :], in_=out_sb[:, :])

            if repeat == 1:
                body()
            else:
                with tc.For_i(0, repeat, 1):
                    body()

            if bench:
                nc.sync.dma_start(out=dummy[:, :], in_=ce_sb[:, :4])

    if DEDUP_LDW:
        _dedup_ldweights(nc)
    _split_waits(nc)
    return nc


def _route(xf, router_w):
    """Host-side router: replicate reference's top-2 + softmax in f64."""
    logits = xf.astype(np.float64) @ router_w.astype(np.float64).T  # [T, E]
    # stable argsort of negated logits == top_k tie-break (lower idx first)
    order = np.argsort(-logits, axis=1, kind="stable")[:, :TOP_K]  # [T, 2]
    top_vals = np.take_along_axis(logits, order, axis=1)
    ex = np.exp(top_vals - top_vals[:, :1])
    top_w = ex / ex.sum(axis=1, keepdims=True)  # [T, 2]
    return order.astype(np.int64), top_w


def kernel(x, router_w, Wg, Wu, Wd):
    from concourse.bass_utils import run_bass_kernel_spmd

    in_dtype = x.dtype
    xf = np.ascontiguousarray(x.reshape(T, H), dtype=np.float32)
    top_idx, top_w = _route(xf, np.asarray(router_w, dtype=np.float32))

    # per-expert token lists
    ids = []
    wts = []
    for e in range(E):
        sel = np.nonzero(top_idx == e)
        ids.append(sel[0])
        wts.append(top_w[sel[0], sel[1]].astype(np.float32))
    counts = np.array([len(a) for a in ids])
    C = max(256, int(np.ceil(counts.max() / 128)) * 128)

    def stripe_pack(w):
        # [I, H] -> [IT, 128p(h%128), HT*128(i-col)] contiguous stripes
        a = np.asarray(w, np.float32).reshape(IT, 128, HT, 128)
        return np.ascontiguousarray(
            a.transpose(0, 3, 2, 1).reshape(IT, 128, H).astype(BF16)
        )

    xf16 = xf.astype(BF16)
    in_maps = []
    for e in range(E):
        n_e = counts[e]
        xT_e = np.zeros((H, C), dtype=BF16)
        xT_e[:, :n_e] = xf16[ids[e]].T
        ce_col = np.zeros(C, dtype=np.float32)
        ce_col[:n_e] = wts[e]
        ce_e = np.ascontiguousarray(ce_col.reshape(C // 128, 128).T)
        in_maps.append(
            {
                "xT": xT_e,
                "wg": stripe_pack(Wg[e]),
                "wu": stripe_pack(Wu[e]),
                "wd": np.ascontiguousarray(
                    np.asarray(Wd[e], np.float32).T.astype(BF16)
                ),
                "ce": ce_e,
            }
        )

    nc = _PROG_CACHE.get(C)
    if nc is None:
        nc = _build_program(C)
        _PROG_CACHE[C] = nc

    res = run_bass_kernel_spmd(nc, in_maps, list(range(E)))

    out = np.zeros((T, H), dtype=np.float32)
    for e in range(E):
        n_e = counts[e]
        np.add.at(out, ids[e],
                  np.asarray(res.results[e]["y"][:n_e], dtype=np.float32))
    return out.reshape(B, S, H).astype(in_dtype, copy=False)
